# revision 1
# baseline (speedup 1.0000x reference)
"""CrissCrossAttention (multi-scale dilated conv + criss-cross axial attention)
Trainium2 Bass/Tile kernel, 8 NeuronCores.

Sharding: 8 cores = 4 batch samples x 2 H-halves. Each core computes the
multi-scale conv (3 dilated 3x3 convs folded into 25 unique sparse taps ->
25 matmul accumulations) for its own 48 rows only (host supplies a 3-row
halo slab), projects q/k/v, then exchanges its half of k and v^T with its
pair-partner core via AllGather so every core has the full column (H)
extent the criss-cross attention needs. The host concatenates the halves.
"""

import numpy as np
import ml_dtypes

BF16 = ml_dtypes.bfloat16

B, C, H, W = 4, 256, 96, 96
CQ = 32
HC = 48            # rows per core
NPOS = HC * W      # 4608 positions per core
HP, WP = HC + 6, W + 6
NCORES = 8
NEG = -1e30

NT_CONV = 12       # 12 N-tiles of 384 (4 image rows each)
CONV_N = 384
NT_PROJ = 9        # 9 N-tiles of 512
PROJ_N = 512


def _fold_taps(w_ms):
    taps = {}
    for i, d in enumerate((1, 2, 3)):
        for iy in range(3):
            for ix in range(3):
                off = ((iy - 1) * d, (ix - 1) * d)
                if off in taps:
                    taps[off] = taps[off] + w_ms[i][:, :, iy, ix]
                else:
                    taps[off] = w_ms[i][:, :, iy, ix].copy()
    offs = sorted(taps)
    assert len(offs) == 25
    return offs, taps


def _build_program(gamma_f, offs):
    import concourse.mybir as mybir
    import concourse.tile as tile
    from concourse import bacc
    from concourse.bass import ts
    from concourse.masks import make_identity

    dt = mybir.dt
    nc = bacc.Bacc("TRN2", target_bir_lowering=False, debug=False,
                   num_devices=NCORES)

    xpad_d = nc.dram_tensor("xpad", [2, 128, HP, WP], dt.bfloat16, kind="ExternalInput")
    w25_d = nc.dram_tensor("w25", [25, 2, 128, 2, 128], dt.bfloat16, kind="ExternalInput")
    wqT_d = nc.dram_tensor("wqT", [2, 128, CQ], dt.bfloat16, kind="ExternalInput")
    wkT_d = nc.dram_tensor("wkT", [2, 128, CQ], dt.bfloat16, kind="ExternalInput")
    wvT_d = nc.dram_tensor("wvT", [2, 128, 256], dt.bfloat16, kind="ExternalInput")
    bq_d = nc.dram_tensor("bq", [CQ, 1], dt.float32, kind="ExternalInput")
    bk_d = nc.dram_tensor("bk", [CQ, 1], dt.float32, kind="ExternalInput")
    bsum_d = nc.dram_tensor("bsum", [2, 128, 1], dt.float32, kind="ExternalInput")
    dmask_d = nc.dram_tensor("dmask", [HC, H], dt.float32, kind="ExternalInput")
    xres_d = nc.dram_tensor("xres", [2, 128, NPOS], dt.float32, kind="ExternalInput")
    out_d = nc.dram_tensor("out", [2, 128, NPOS], dt.float32, kind="ExternalOutput")

    with tile.TileContext(nc) as tc:
        with (
            tc.tile_pool(name="const", bufs=1) as constp,
            tc.tile_pool(name="dram", bufs=1, space="DRAM") as dramp,
            tc.tile_pool(name="accp", bufs=1) as accp,
            tc.tile_pool(name="attp", bufs=1) as attp,
            tc.tile_pool(name="midp", bufs=1) as midp,
            tc.tile_pool(name="smallp", bufs=1) as smallp,
        ):
            # ---- constants ----
            id_bf = constp.tile([128, 128], dt.bfloat16, tag="idbf", name="id_bf")
            make_identity(nc, id_bf)
            id_f32 = constp.tile([96, 96], dt.float32, tag="idf32", name="id_f32")
            make_identity(nc, id_f32)
            bq_sb = constp.tile([CQ, 1], dt.float32, tag="bq", name="bq_sb")
            nc.sync.dma_start(out=bq_sb, in_=bq_d[:])
            bk_sb = constp.tile([CQ, 1], dt.float32, tag="bk", name="bk_sb")
            nc.sync.dma_start(out=bk_sb, in_=bk_d[:])
            bsum_sb = [constp.tile([128, 1], dt.float32, tag=f"bs{m}", name=f"bsum{m}")
                       for m in range(2)]
            for m in range(2):
                nc.sync.dma_start(out=bsum_sb[m], in_=bsum_d[m])
            dmask_sb = constp.tile([HC, H], dt.float32, tag="dm", name="dmask_sb")
            nc.sync.dma_start(out=dmask_sb, in_=dmask_d[:])

            # ---- persistent tensors ----
            acc = [accp.tile([128, NPOS], dt.bfloat16, tag=f"acc{m}", name=f"acc{m}")
                   for m in range(2)]
            attH = attp.tile([HC, W, H], dt.bfloat16, tag="attH", name="attH")
            attW = attp.tile([W, HC, W], dt.bfloat16, tag="attW", name="attW")
            q_sb = midp.tile([CQ, NPOS], dt.bfloat16, tag="q", name="q_sb")
            k_sb = midp.tile([CQ, NPOS], dt.bfloat16, tag="k", name="k_sb")
            v_hw = [midp.tile([128, NPOS], dt.bfloat16, tag=f"v{m}", name=f"vhw{m}")
                    for m in range(2)]
            sH = smallp.tile([HC, W], dt.float32, tag="sH", name="sH")
            sW = smallp.tile([W, HC], dt.float32, tag="sWt", name="sW")
            s_h = smallp.tile([HC, W], dt.float32, tag="s_h", name="s_h")
            recip_h = smallp.tile([HC, W], dt.float32, tag="rh", name="recip_h")
            recip_w = smallp.tile([W, HC], dt.float32, tag="rw", name="recip_w")

            # ---- dram bounce buffers for the pair exchange ----
            pack_k = dramp.tile([CQ, NPOS], dt.bfloat16, tag="pk", name="pack_k")
            pack_v = dramp.tile([W, HC, 256], dt.bfloat16, tag="pv", name="pack_v")
            gath_k = dramp.tile([2, CQ, NPOS], dt.bfloat16, tag="gk", name="gath_k")
            gath_v = dramp.tile([2, W, HC, 256], dt.bfloat16, tag="gv", name="gath_v")

            with tc.tile_pool(name="msp", bufs=1) as msp:
                ms_hw = [msp.tile([128, NPOS], dt.bfloat16, tag=f"ms{m}", name=f"ms{m}")
                         for m in range(2)]

                # ================= Phase 1: conv (25 taps) =================
                with (
                    tc.tile_pool(name="xw", bufs=1) as xwp,
                    tc.tile_pool(name="cvps", bufs=1, space="PSUM") as cvps,
                ):
                    xpad_sb = [xwp.tile([128, HP, WP], dt.bfloat16, tag=f"xp{k}",
                                        name=f"xp{k}") for k in range(2)]
                    for k in range(2):
                        nc.sync.dma_start(out=xpad_sb[k], in_=xpad_d[k])
                    w25_sb = [xwp.tile([128, 25, 2, 128], dt.bfloat16, tag=f"wt{k}",
                                       name=f"w25{k}") for k in range(2)]
                    for k in range(2):
                        nc.sync.dma_start(out=w25_sb[k],
                                          in_=w25_d[:, k].rearrange("t p m c -> p t m c"))

                    for g in range(3):      # 3 groups of 4 N-tiles -> 8 psum banks
                        P = [[cvps.tile([128, CONV_N], dt.float32, tag=f"cv{m}{j}",
                                        name=f"P{g}{m}{j}", bufs=1)
                              for j in range(4)] for m in range(2)]
                        for t in range(25):
                            dy, dx = offs[t]
                            for k in range(2):
                                first = (t == 0 and k == 0)
                                last = (t == 24 and k == 1)
                                for m in range(2):
                                    lhsT = w25_sb[k][:, t, m, :]
                                    for j in range(4):
                                        nj = g * 4 + j
                                        rhs = xpad_sb[k][:, nj * 4 + 3 + dy: nj * 4 + 7 + dy,
                                                         3 + dx: 3 + dx + W]
                                        nc.tensor.matmul(P[m][j], lhsT, rhs,
                                                         start=first, stop=last)
                        for m in range(2):
                            for j in range(4):
                                nj = g * 4 + j
                                nc.vector.tensor_scalar_add(
                                    out=ms_hw[m][:, nj * CONV_N:(nj + 1) * CONV_N],
                                    in0=P[m][j], scalar1=bsum_sb[m])

                ms3 = [ms_hw[k].rearrange("p (h w) -> p h w", w=W) for k in range(2)]

                # ======== Phase 2: projections + pair exchange ========
                with (
                    tc.tile_pool(name="pjps", bufs=1, space="PSUM") as pjps,
                    tc.tile_pool(name="pjcp", bufs=1) as pjcp,
                    tc.tile_pool(name="wproj", bufs=1) as wpp,
                ):
                    wqT_sb = [wpp.tile([128, CQ], dt.bfloat16, tag=f"wq{k}",
                                       name=f"wq{k}") for k in range(2)]
                    wkT_sb = [wpp.tile([128, CQ], dt.bfloat16, tag=f"wk{k}",
                                       name=f"wk{k}") for k in range(2)]
                    wvT_sb = [wpp.tile([128, 256], dt.bfloat16, tag=f"wv{k}",
                                       name=f"wv{k}") for k in range(2)]
                    for k in range(2):
                        nc.sync.dma_start(out=wqT_sb[k], in_=wqT_d[k])
                        nc.sync.dma_start(out=wkT_sb[k], in_=wkT_d[k])
                        nc.sync.dma_start(out=wvT_sb[k], in_=wvT_d[k])

                    # k projection first so its exchange starts early
                    for n in range(NT_PROJ):
                        sl = slice(n * PROJ_N, (n + 1) * PROJ_N)
                        pk = pjps.tile([CQ, PROJ_N], dt.float32, tag="pq",
                                       name=f"pk{n}", bufs=2)
                        for k in range(2):
                            nc.tensor.matmul(pk, wkT_sb[k], ms_hw[k][:, sl],
                                             start=(k == 0), stop=(k == 1))
                        nc.vector.tensor_scalar_add(out=k_sb[:, sl], in0=pk,
                                                    scalar1=bk_sb)
                    nc.gpsimd.dma_start(out=pack_k[:], in_=k_sb[:])
                    nc.gpsimd.collective_compute(
                        "AllGather", mybir.AluOpType.bypass,
                        replica_groups=[[0, 1], [2, 3], [4, 5], [6, 7]],
                        ins=[pack_k[:]], outs=[gath_k[:]])

                    # v^T (own half, w-major chunks) -> pack for exchange
                    for w in range(W):
                        pvt = pjps.tile([HC, 256], dt.float32, tag="pvt",
                                        name=f"pvt{w}", bufs=4)
                        for k in range(2):
                            nc.tensor.matmul(pvt, ms3[k][:, :, w], wvT_sb[k],
                                             start=(k == 0), stop=(k == 1))
                        stg = pjcp.tile([HC, 256], dt.bfloat16, tag="stg",
                                        name=f"stg{w}", bufs=6)
                        nc.vector.tensor_copy(out=stg, in_=pvt)
                        nc.gpsimd.dma_start(out=pack_v[w], in_=stg)
                    nc.gpsimd.collective_compute(
                        "AllGather", mybir.AluOpType.bypass,
                        replica_groups=[[0, 1], [2, 3], [4, 5], [6, 7]],
                        ins=[pack_v[:]], outs=[gath_v[:]])

                    # q projection
                    for n in range(NT_PROJ):
                        sl = slice(n * PROJ_N, (n + 1) * PROJ_N)
                        pq = pjps.tile([CQ, PROJ_N], dt.float32, tag="pq",
                                       name=f"pq{n}", bufs=2)
                        for k in range(2):
                            nc.tensor.matmul(pq, wqT_sb[k], ms_hw[k][:, sl],
                                             start=(k == 0), stop=(k == 1))
                        nc.vector.tensor_scalar_add(out=q_sb[:, sl], in0=pq,
                                                    scalar1=bq_sb)

                    # v (own half, h-major layout, no bias)
                    for m in range(2):
                        for n in range(NT_PROJ):
                            sl = slice(n * PROJ_N, (n + 1) * PROJ_N)
                            pv = pjps.tile([128, PROJ_N], dt.float32, tag="pv",
                                           name=f"pv{m}{n}", bufs=2)
                            for k in range(2):
                                nc.tensor.matmul(pv, wvT_sb[k][:, m * 128:(m + 1) * 128],
                                                 ms_hw[k][:, sl],
                                                 start=(k == 0), stop=(k == 1))
                            nc.vector.tensor_copy(out=v_hw[m][:, sl], in_=pv)
            # msp released here (frees ms before the big attention tensors)

            q3 = q_sb.rearrange("p (h w) -> p h w", w=W)
            k3 = k_sb.rearrange("p (h w) -> p h w", w=W)

            # ================= Phase 3: energies + exp =================
            with tc.tile_pool(name="gat", bufs=1) as gatp:
              with (
                tc.tile_pool(name="enps", bufs=1, space="PSUM") as enps,
                tc.tile_pool(name="encp", bufs=1) as encp,
              ):
                  # row (W) energies first: only need own-half q/k
                  for h in range(HC):
                      pew = enps.tile([W, W], dt.float32, tag="ew", name=f"ew{h}", bufs=3)
                      nc.tensor.matmul(pew, q3[:, h, :], k3[:, h, :], start=True, stop=True)
                      nc.scalar.activation(out=attW[:, h, :], in_=pew,
                                           func=mybir.ActivationFunctionType.Exp,
                                           accum_out=sW[:, h:h + 1])

                  # assemble full-H k and v^T from the gathered halves
                  k_full = gatp.tile([CQ, W, H], dt.bfloat16, tag="kf", name="k_full")
                  for gi in range(2):
                      ko = encp.tile([CQ, NPOS], dt.bfloat16, tag="ko",
                                     name=f"ko{gi}", bufs=2)
                      nc.sync.dma_start(out=ko, in_=gath_k[gi])
                      nc.vector.tensor_copy(
                          out=k_full[:, :, gi * HC:(gi + 1) * HC],
                          in_=ko.rearrange("p (h w) -> p w h", w=W))
                  vT_wo = gatp.tile([H, W, 256], dt.bfloat16, tag="vt", name="vT_wo")
                  for gi in range(2):
                      nc.sync.dma_start(
                          out=vT_wo[gi * HC:(gi + 1) * HC],
                          in_=gath_v[gi].rearrange("w h c -> h w c"))

                  # column (H) energies with diagonal mask
                  for w in range(W):
                      peh = enps.tile([HC, H], dt.float32, tag="eh", name=f"eh{w}", bufs=3)
                      nc.tensor.matmul(peh, q3[:, :, w], k_full[:, w, :],
                                       start=True, stop=True)
                      ehm = encp.tile([HC, H], dt.float32, tag="ehm",
                                      name=f"ehm{w}", bufs=6)
                      nc.vector.tensor_add(out=ehm, in0=peh, in1=dmask_sb)
                      nc.scalar.activation(out=attH[:, w, :], in_=ehm,
                                           func=mybir.ActivationFunctionType.Exp,
                                           accum_out=sH[:, w:w + 1])

                  # joint softmax denominator and reciprocals
                  pt1 = enps.tile([HC, W], dt.float32, tag="tr", name="pt1", bufs=1)
                  nc.tensor.transpose(pt1, sW, id_f32)
                  nc.vector.tensor_add(out=s_h, in0=sH, in1=pt1)
                  nc.vector.reciprocal(out=recip_h, in_=s_h)
                  pt2 = enps.tile([W, HC], dt.float32, tag="tr2", name="pt2", bufs=1)
                  nc.tensor.transpose(pt2, recip_h, id_f32[0:HC, 0:HC])
                  nc.vector.tensor_copy(out=recip_w, in_=pt2)

              # ======== Phase 4a: row attention application ========
              with (
                  tc.tile_pool(name="apps", bufs=1, space="PSUM") as apps,
                  tc.tile_pool(name="appc", bufs=1) as appc,
              ):
                  from concourse.bass import ts as _ts
                  for h in range(HC):
                      awn = appc.tile([W, W], dt.bfloat16, tag="awn",
                                      name=f"awn{h}", bufs=6)
                      nc.vector.tensor_scalar_mul(out=awn, in0=attW[:, h, :],
                                                  scalar1=recip_w[:, h:h + 1])
                      ptw = apps.tile([W, W], dt.bfloat16, tag="tw",
                                      name=f"ptw{h}", bufs=2)
                      nc.tensor.transpose(ptw, awn, id_bf[0:W, 0:W])
                      awnT = appc.tile([W, W], dt.bfloat16, tag="awnT",
                                       name=f"awnT{h}", bufs=6)
                      nc.vector.tensor_copy(out=awnT, in_=ptw)
                      for m in range(2):
                          ptv = apps.tile([W, 128], dt.bfloat16, tag="tv",
                                          name=f"ptv{h}{m}", bufs=3)
                          nc.tensor.transpose(ptv, v_hw[m][:, _ts(h, W)], id_bf)
                          vTr = appc.tile([W, 128], dt.bfloat16, tag="vTr",
                                          name=f"vTr{h}{m}", bufs=6)
                          nc.vector.tensor_copy(out=vTr, in_=ptv)
                          po = apps.tile([128, W], dt.float32, tag="po",
                                         name=f"po{h}{m}", bufs=3)
                          nc.tensor.matmul(po, vTr, awnT, start=True, stop=True)
                          nc.vector.tensor_copy(out=acc[m][:, _ts(h, W)], in_=po)

              # ======== Phase 4b: column attention application ========
              acc3 = [acc[m].rearrange("p (h w) -> p h w", w=W) for m in range(2)]
              with (
                  tc.tile_pool(name="apps2", bufs=1, space="PSUM") as apps2,
                  tc.tile_pool(name="appc2", bufs=1) as appc2,
              ):
                  for w in range(W):
                      ahn = appc2.tile([HC, H], dt.bfloat16, tag="ahn",
                                       name=f"ahn{w}", bufs=6)
                      nc.vector.tensor_scalar_mul(out=ahn, in0=attH[:, w, :],
                                                  scalar1=recip_h[:, w:w + 1])
                      pth = apps2.tile([H, HC], dt.bfloat16, tag="th",
                                       name=f"pth{w}", bufs=4)
                      nc.tensor.transpose(pth, ahn, id_bf[0:HC, 0:HC])
                      ahnT = appc2.tile([H, HC], dt.bfloat16, tag="ahnT",
                                        name=f"ahnT{w}", bufs=6)
                      nc.vector.tensor_copy(out=ahnT, in_=pth)
                      for m in range(2):
                          po2 = apps2.tile([128, HC], dt.float32, tag="po2",
                                           name=f"po2{w}{m}", bufs=4)
                          nc.tensor.matmul(po2, vT_wo[:, w, m * 128:(m + 1) * 128],
                                           ahnT, start=True, stop=True)
                          nc.vector.tensor_add(out=acc3[m][:, :, w],
                                               in0=acc3[m][:, :, w], in1=po2)

              # ======== Phase 5: residual + output ========
              with tc.tile_pool(name="fin", bufs=1) as finp:
                  for m in range(2):
                      for n in range(NT_PROJ):
                          sl = slice(n * PROJ_N, (n + 1) * PROJ_N)
                          xr = finp.tile([128, PROJ_N], dt.float32, tag="xr",
                                         name=f"xr{m}{n}", bufs=3)
                          nc.sync.dma_start(out=xr, in_=xres_d[m][:, sl])
                          fo = finp.tile([128, PROJ_N], dt.float32, tag="fo",
                                         name=f"fo{m}{n}", bufs=3)
                          nc.vector.scalar_tensor_tensor(
                              out=fo, in0=acc[m][:, sl], scalar=float(gamma_f),
                              in1=xr, op0=mybir.AluOpType.mult,
                              op1=mybir.AluOpType.add)
                          nc.sync.dma_start(out=out_d[m][:, sl], in_=fo)

    nc.compile()
    return nc


def _prepare_inputs(x, w_ms, b_ms, wq, bq, wk, bk, wv, bv, gamma):
    offs, taps = _fold_taps(np.asarray(w_ms, np.float32))
    x = np.asarray(x, np.float32)
    bsum = np.asarray(b_ms, np.float32).sum(0)
    gamma_f = float(np.asarray(gamma))
    bv = np.asarray(bv, np.float32)

    w25 = np.empty((25, 2, 128, 2, 128), np.float32)
    for t, off in enumerate(offs):
        w25[t] = taps[off].T.reshape(2, 128, 2, 128)   # [ci, co] chunked
    w25 = w25.astype(BF16)
    wqT = np.asarray(wq, np.float32).T.reshape(2, 128, CQ).astype(BF16)
    wkT = np.asarray(wk, np.float32).T.reshape(2, 128, CQ).astype(BF16)
    wvT = np.asarray(wv, np.float32).T.reshape(2, 128, 256).astype(BF16)
    bq_a = np.ascontiguousarray(np.asarray(bq, np.float32).reshape(CQ, 1))
    bk_a = np.ascontiguousarray(np.asarray(bk, np.float32).reshape(CQ, 1))
    bsum_a = np.ascontiguousarray(bsum.reshape(2, 128, 1))

    in_maps = []
    for core in range(NCORES):
        b, g = core // 2, core % 2
        h0 = g * HC
        xp = np.zeros((C, H + 6, W + 6), np.float32)
        xp[:, 3:3 + H, 3:3 + W] = x[b]
        xpad = np.ascontiguousarray(
            xp[:, h0:h0 + HP, :]).reshape(2, 128, HP, WP).astype(BF16)
        dmask = np.zeros((HC, H), np.float32)
        dmask[np.arange(HC), h0 + np.arange(HC)] = NEG
        xres = (x[b, :, h0:h0 + HC, :].reshape(C, NPOS)
                + gamma_f * bv[:, None]).reshape(2, 128, NPOS)
        in_maps.append({
            "xpad": xpad, "w25": w25, "wqT": wqT, "wkT": wkT, "wvT": wvT,
            "bq": bq_a, "bk": bk_a, "bsum": bsum_a, "dmask": dmask,
            "xres": np.ascontiguousarray(xres.astype(np.float32)),
        })
    return in_maps, gamma_f, offs


def run(inputs, trace=False):
    from concourse.bass_utils import run_bass_kernel_spmd
    in_maps, gamma_f, offs = _prepare_inputs(**inputs)
    nc = _build_program(gamma_f, offs)
    res = run_bass_kernel_spmd(nc, in_maps, list(range(NCORES)), trace=trace)
    out = np.empty((B, C, H, W), np.float32)
    for core in range(NCORES):
        b, g = core // 2, core % 2
        r = np.asarray(res.results[core]["out"]).reshape(C, HC, W)
        out[b, :, g * HC:(g + 1) * HC, :] = r
    return out, res


def kernel(**inputs) -> np.ndarray:
    out, _ = run(inputs, trace=False)
    return out



# revision 11
# speedup vs baseline: 1.6257x; 1.6257x over previous
"""CrissCrossAttention (multi-scale dilated conv + criss-cross axial attention)
Trainium2 Bass/Tile kernel, 8 NeuronCores.

Sharding: 8 cores = 4 batch samples x 2 H-halves.

v2 design:
 - conv as 25 folded taps in fp8e4 with DoubleRow (K=256 per matmul), flat
   416-col rhs runs spanning 4 padded rows (halo cols never read back).
 - all projections (q/k/vT) in fp8 DoubleRow off an fp8 ms tensor.
 - energies computed TRANSPOSED (source index on partitions) so no per-line
   transposes are needed; exp batched 5-10 lines per scalar-engine call.
 - softmax denominators ride along as a ones-column appended to the vT
   operands of the apply matmuls; joint normalization deferred to the tail.
 - pair exchange: k (bf16) and vT (fp8) via AllGather, produced right after
   conv so the transfers hide behind local row attention.
 - output kept in [w, h, c] layout on chip; host transposes for free.
"""

import numpy as np
import ml_dtypes

BF16 = ml_dtypes.bfloat16
F8 = ml_dtypes.float8_e4m3

B, C, H, W = 4, 256, 96, 96
CQ = 32
HC = 48              # rows per core
NPOS = HC * W        # 4608 positions per core
HP, WP = 58, 104     # padded slab: 3+48+3 halo rows +4 slack, 3+96+5 cols
NCORES = 8
SW = 64.0            # weight scale for fp8
SV = 32.0            # v scale for fp8

NROW = 4             # image rows per conv N-tile
CONV_N = NROW * WP   # 416 flat cols per conv matmul (incl junk)
NT_PROJ = 9
PROJ_N = 512


def _fold_taps(w_ms):
    taps = {}
    for i, d in enumerate((1, 2, 3)):
        for iy in range(3):
            for ix in range(3):
                off = ((iy - 1) * d, (ix - 1) * d)
                if off in taps:
                    taps[off] = taps[off] + w_ms[i][:, :, iy, ix]
                else:
                    taps[off] = w_ms[i][:, :, iy, ix].copy()
    offs = sorted(taps)
    assert len(offs) == 25
    return offs, taps


def _build_program(gamma_f, offs):
    import concourse.mybir as mybir
    import concourse.tile as tile
    from concourse import bacc
    from concourse.masks import make_identity

    dt = mybir.dt
    DR = mybir.MatmulPerfMode.DoubleRow
    nc = bacc.Bacc("TRN2", target_bir_lowering=False, debug=False,
                   num_devices=NCORES)

    xpad_d = nc.dram_tensor("xpad", [128, 2, HP, WP], dt.float8e4, kind="ExternalInput")
    w25_d = nc.dram_tensor("w25", [128, 25, 2, 2, 128], dt.float8e4, kind="ExternalInput")
    wq_d = nc.dram_tensor("wq8", [128, 2, CQ], dt.float8e4, kind="ExternalInput")
    wk_d = nc.dram_tensor("wk8", [128, 2, CQ], dt.float8e4, kind="ExternalInput")
    wv_d = nc.dram_tensor("wv8", [128, 2, 256], dt.float8e4, kind="ExternalInput")
    bq_d = nc.dram_tensor("bq", [CQ, 1], dt.float32, kind="ExternalInput")
    bk_d = nc.dram_tensor("bk", [CQ, 1], dt.float32, kind="ExternalInput")
    bsum_d = nc.dram_tensor("bsum", [2, 128, 1], dt.float32, kind="ExternalInput")
    mask_d = nc.dram_tensor("mask01", [96, NPOS], dt.bfloat16, kind="ExternalInput")
    xres_d = nc.dram_tensor("xresT", [96, HC, 256], dt.float32, kind="ExternalInput")
    out_d = nc.dram_tensor("out", [96, HC, 256], dt.float32, kind="ExternalOutput")

    with tile.TileContext(nc) as tc:
        with (
            tc.tile_pool(name="const", bufs=1) as constp,
            tc.tile_pool(name="dram", bufs=1, space="DRAM") as dramp,
            tc.tile_pool(name="persist", bufs=1) as pp,
        ):
            # ---- constants ----
            id48 = constp.tile([HC, HC], dt.bfloat16, tag="id48", name="id48")
            make_identity(nc, id48)
            bq_sb = constp.tile([CQ, 1], dt.float32, tag="bq", name="bq_sb")
            nc.sync.dma_start(out=bq_sb, in_=bq_d[:])
            bk_sb = constp.tile([CQ, 1], dt.float32, tag="bk", name="bk_sb")
            nc.sync.dma_start(out=bk_sb, in_=bk_d[:])
            bsum_sb = [constp.tile([128, 1], dt.float32, tag=f"bs{m}", name=f"bsum{m}")
                       for m in range(2)]
            for m in range(2):
                nc.sync.dma_start(out=bsum_sb[m], in_=bsum_d[m])
            mask_sb = constp.tile([96, NPOS], dt.bfloat16, tag="msk", name="mask_sb")
            nc.sync.dma_start(out=mask_sb, in_=mask_d[:])

            # ---- persistent tensors ----
            kf = pp.tile([CQ, 2, NPOS], dt.bfloat16, tag="kf", name="kf")
            q_sb = pp.tile([CQ, NPOS], dt.bfloat16, tag="q", name="q_sb")
            vTa = pp.tile([96, HC, 257], dt.float8e4, tag="vTa", name="vTa")
            vTb = pp.tile([96, 96, 257], dt.float8e4, tag="vTb", name="vTb")
            accR = pp.tile([96, HC, 257], dt.bfloat16, tag="accR", name="accR")
            accC = pp.tile([HC, 96, 257], dt.bfloat16, tag="accC", name="accC")
            recipD = pp.tile([96, HC], dt.float32, tag="rD", name="recipD")

            # ones columns for the denominator trick
            nc.vector.memset(vTa[:, :, 256], 1.0)
            nc.vector.memset(vTb[:, :, 256], 1.0)

            # ---- dram bounce buffers for the pair exchange ----
            pack_k = dramp.tile([CQ, NPOS], dt.bfloat16, tag="pk", name="pack_k")
            pack_v = dramp.tile([HC, 96, 256], dt.float8e4, tag="pv", name="pack_v")
            gath_k = dramp.tile([2, CQ, NPOS], dt.bfloat16, tag="gk", name="gath_k")
            gath_v = dramp.tile([2, HC, 96, 256], dt.float8e4, tag="gv", name="gath_v")
            bC_d = dramp.tile([HC, 96, 256], dt.bfloat16, tag="bC", name="bC_d")

            # Own k is written into plane 0 of kf (row attention reads it
            # there); after the AllGather both planes are overwritten with the
            # globally-indexed halves (group rank == global half index), which
            # is what column attention needs.  No per-core branching required.

            # ================= Phase 1: conv (25 taps, fp8 DoubleRow) ========
            msp_ctx = tc.tile_pool(name="msp", bufs=1)
            msp = msp_ctx.__enter__()
            ms8 = msp.tile([128, 2, NPOS], dt.float8e4, tag="ms8", name="ms8")
            with (
                tc.tile_pool(name="xw", bufs=1) as xwp,
                tc.tile_pool(name="cvps", bufs=1, space="PSUM") as cvps,
            ):
                w25_sb = xwp.tile([128, 25, 2, 2, 128], dt.float8e4, tag="wt",
                                  name="w25_sb")
                nc.sync.dma_start(out=w25_sb, in_=w25_d[:])
                xpad_sb = xwp.tile([128, 2, HP, WP], dt.float8e4, tag="xp",
                                   name="xpad_sb")
                nc.sync.dma_start(out=xpad_sb, in_=xpad_d[:])
                xflat = xpad_sb.rearrange("p k h w -> p k (h w)")

                for g in range(3):      # 3 groups of (2m x 4j) psum tiles
                    P = [[cvps.tile([128, CONV_N], dt.float32, tag=f"cv{m}{j}",
                                    name=f"P{g}{m}{j}", bufs=1)
                          for j in range(4)] for m in range(2)]
                    for t in range(25):
                        dy, dx = offs[t]
                        for m in range(2):
                            lhsT = w25_sb[:, t, :, m, :]
                            for j in range(4):
                                nj = g * 4 + j
                                base = (nj * NROW + 3 + dy) * WP + 3 + dx
                                rhs = xflat[:, :, base:base + CONV_N]
                                nc.tensor.matmul(P[m][j], lhsT, rhs,
                                                 start=(t == 0), stop=(t == 24),
                                                 perf_mode=DR)
                    for m in range(2):
                        for j in range(4):
                            nj = g * 4 + j
                            dst = ms8[:, m, nj * NROW * W:(nj + 1) * NROW * W]
                            nc.vector.tensor_scalar(
                                out=dst.rearrange("p (r c) -> p r c", c=W),
                                in0=P[m][j].rearrange("p (r c) -> p r c", c=WP)[:, :, 0:W],
                                scalar1=1.0 / SW, scalar2=bsum_sb[m],
                                op0=mybir.AluOpType.mult,
                                op1=mybir.AluOpType.add)

            # ================= Phase 2: projections (fp8 DoubleRow) =========
            with (
                tc.tile_pool(name="wproj", bufs=1) as wpp,
                tc.tile_pool(name="pjps", bufs=1, space="PSUM") as pjps,
            ):
                wq_sb = wpp.tile([128, 2, CQ], dt.float8e4, tag="wq", name="wq_sb")
                nc.sync.dma_start(out=wq_sb, in_=wq_d[:])
                wk_sb = wpp.tile([128, 2, CQ], dt.float8e4, tag="wk", name="wk_sb")
                nc.sync.dma_start(out=wk_sb, in_=wk_d[:])
                wv_sb = wpp.tile([128, 2, 256], dt.float8e4, tag="wv", name="wv_sb")
                nc.sync.dma_start(out=wv_sb, in_=wv_d[:])

                # k projection first so its exchange starts early.
                # own k goes into BOTH planes of kf (partner plane is
                # overwritten after the gather).
                for n in range(NT_PROJ):
                    sl = slice(n * PROJ_N, (n + 1) * PROJ_N)
                    pk = pjps.tile([CQ, PROJ_N], dt.float32, tag="pjk",
                                   name=f"pk{n}", bufs=2)
                    nc.tensor.matmul(pk, wk_sb, ms8[:, :, sl], start=True,
                                     stop=True, perf_mode=DR)
                    nc.vector.tensor_scalar(out=kf[:, 0, sl], in0=pk,
                                            scalar1=1.0 / SW, scalar2=bk_sb,
                                            op0=mybir.AluOpType.mult,
                                            op1=mybir.AluOpType.add)
                nc.gpsimd.dma_start(out=pack_k[:], in_=kf[:, 0, :])
                nc.gpsimd.collective_compute(
                    "AllGather", mybir.AluOpType.bypass,
                    replica_groups=[[0, 1], [2, 3], [4, 5], [6, 7]],
                    ins=[pack_k[:]], outs=[gath_k[:]])

                # vT (own half): per h, out [96 w', 256 c]
                for h in range(HC):
                    pv = pjps.tile([96, 256], dt.float32, tag="pjv",
                                   name=f"pv{h}", bufs=2)
                    nc.tensor.matmul(pv, ms8[:, :, h * W:(h + 1) * W], wv_sb,
                                     start=True, stop=True, perf_mode=DR)
                    nc.vector.tensor_scalar_mul(out=vTa[:, h, 0:256], in0=pv,
                                                scalar1=SV / SW)
                nc.gpsimd.dma_start(
                    out=pack_v[:].rearrange("h w c -> w h c"),
                    in_=vTa[:, :, 0:256])
                nc.gpsimd.collective_compute(
                    "AllGather", mybir.AluOpType.bypass,
                    replica_groups=[[0, 1], [2, 3], [4, 5], [6, 7]],
                    ins=[pack_v[:]], outs=[gath_v[:]])

                # q projection
                for n in range(NT_PROJ):
                    sl = slice(n * PROJ_N, (n + 1) * PROJ_N)
                    pq = pjps.tile([CQ, PROJ_N], dt.float32, tag="pjq",
                                   name=f"pq{n}", bufs=2)
                    nc.tensor.matmul(pq, wq_sb, ms8[:, :, sl], start=True,
                                     stop=True, perf_mode=DR)
                    nc.vector.tensor_scalar(out=q_sb[:, sl], in0=pq,
                                            scalar1=1.0 / SW, scalar2=bq_sb,
                                            op0=mybir.AluOpType.mult,
                                            op1=mybir.AluOpType.add)

            msp_ctx.__exit__(None, None, None)

            # ============ Phase 3: row attention (fully local) ==============
            HB = [5] * 9 + [3]          # 48 h in blocks of 5 (plus tail 3)
            with (
                tc.tile_pool(name="ph3p", bufs=1) as ph3p,
                tc.tile_pool(name="rps", bufs=1, space="PSUM") as rps,
            ):
                attWT = ph3p.tile([96, NPOS], dt.bfloat16, tag="awt",
                                  name="attWT")
                h = 0
                for nb in HB:
                    pew = rps.tile([96, nb * 96], dt.float32, tag="pew",
                                   name=f"pew{h}", bufs=2)
                    for i in range(nb):
                        sl = slice((h + i) * 96, (h + i + 1) * 96)
                        nc.tensor.matmul(pew[:, i * 96:(i + 1) * 96],
                                         kf[:, 0, sl], q_sb[:, sl],
                                         start=True, stop=True)
                    nc.scalar.activation(
                        out=attWT[:, h * 96:(h + nb) * 96], in_=pew,
                        func=mybir.ActivationFunctionType.Exp)
                    h += nb
                for h in range(HC):
                    po = rps.tile([96, 257], dt.float32, tag="po",
                                  name=f"po{h}", bufs=4)
                    nc.tensor.matmul(po, attWT[:, h * 96:(h + 1) * 96],
                                     vTa[:, h, :], start=True, stop=True)
                    nc.vector.tensor_copy(out=accR[:, h, :], in_=po)

            # ============ Phase 4: column attention (needs exchange) ========
            q3 = q_sb.rearrange("p (h w) -> p h w", w=W)
            kf4 = kf.rearrange("p g (h w) -> p g h w", w=W)
            with (
                tc.tile_pool(name="ph4p", bufs=1) as ph4p,
                tc.tile_pool(name="cps", bufs=1, space="PSUM") as cps,
            ):
                attHT = ph4p.tile([96, NPOS], dt.bfloat16, tag="aht",
                                  name="attHT")
                # partner k -> the other plane of kf.  The gathered tensor is
                # indexed by group rank == global half, so copy BOTH planes:
                # own plane gets rewritten with identical data.
                for gi in range(2):
                    nc.sync.dma_start(out=kf[:, gi, :], in_=gath_k[gi])
                # full-H vT for col apply, partition = global H'
                for gi in range(2):
                    nc.sync.dma_start(
                        out=vTb[gi * HC:(gi + 1) * HC, :, 0:256],
                        in_=gath_v[gi])

                WB = [10] * 9 + [6]     # 96 w in blocks of 10 (tail 6)
                w = 0
                for nb in WB:
                    peh = cps.tile([96, nb * HC], dt.float32, tag="peh",
                                   name=f"peh{w}", bufs=2)
                    for i in range(nb):
                        nc.tensor.matmul(peh[:, i * HC:(i + 1) * HC],
                                         kf4[:, :, :, w + i], q3[:, :, w + i],
                                         start=True, stop=True)
                    nc.scalar.activation(
                        out=attHT[:, w * HC:(w + nb) * HC], in_=peh,
                        func=mybir.ActivationFunctionType.Exp)
                    w += nb
                # zero the masked diagonal (att_H has -inf on H'==h)
                nc.vector.tensor_mul(out=attHT[:], in0=attHT[:], in1=mask_sb)
                for w in range(96):
                    po2 = cps.tile([HC, 257], dt.float32, tag="po2",
                                   name=f"po2{w}", bufs=4)
                    nc.tensor.matmul(po2, attHT[:, w * HC:(w + 1) * HC],
                                     vTb[:, w, :], start=True, stop=True)
                    nc.vector.tensor_copy(out=accC[:, w, :], in_=po2)

            # ============ Phase 5: merge, normalize, residual, out ==========
            with (
                tc.tile_pool(name="fin", bufs=1) as finp,
                tc.tile_pool(name="fps", bufs=1, space="PSUM") as fps,
            ):
                accCw = finp.tile([96, HC, 256], dt.bfloat16, tag="accCw",
                                  name="accCw")
                # col results into row layout (partition shift via a DRAM
                # bounce; the permutation lives on the DRAM-side APs)
                nc.sync.dma_start(out=bC_d[:], in_=accC[:, :, 0:256])
                nc.sync.dma_start(
                    out=accCw[:], in_=bC_d[:].rearrange("h w c -> w h c"))
                # joint denominator: D = rowD + colD^T
                ptD = fps.tile([96, HC], dt.bfloat16, tag="ptD", name="ptD",
                               bufs=1)
                nc.tensor.transpose(ptD, accC[:, :, 256], id48)
                Dt = finp.tile([96, HC], dt.float32, tag="Dt", name="Dt")
                nc.vector.tensor_add(out=Dt, in0=accR[:, :, 256], in1=ptD)
                nc.vector.tensor_scalar_mul(out=Dt, in0=Dt,
                                            scalar1=SV / gamma_f)
                nc.vector.reciprocal(out=recipD, in_=Dt)

                NH = 6
                for cidx in range(HC // NH):
                    hsl = slice(cidx * NH, (cidx + 1) * NH)
                    xr = finp.tile([96, NH, 256], dt.float32, tag="xr",
                                   name=f"xr{cidx}", bufs=2)
                    nc.sync.dma_start(out=xr, in_=xres_d[:, hsl, :])
                    nc.vector.tensor_add(out=accCw[:, hsl, :],
                                         in0=accCw[:, hsl, :],
                                         in1=accR[:, hsl, 0:256])
                    for i in range(NH):
                        h = cidx * NH + i
                        if i % 2 == 0:
                            nc.vector.tensor_scalar_mul(
                                out=accCw[:, h, :], in0=accCw[:, h, :],
                                scalar1=recipD[:, h:h + 1])
                        else:
                            nc.scalar.activation(
                                out=accCw[:, h, :], in_=accCw[:, h, :],
                                func=mybir.ActivationFunctionType.Copy,
                                scale=recipD[:, h:h + 1])
                    fo = finp.tile([96, NH, 256], dt.float32, tag="fo",
                                   name=f"fo{cidx}", bufs=2)
                    nc.vector.tensor_add(out=fo, in0=accCw[:, hsl, :], in1=xr)
                    nc.sync.dma_start(out=out_d[:, hsl, :], in_=fo)

    nc.compile()
    return nc


def _prepare_inputs(x, w_ms, b_ms, wq, bq, wk, bk, wv, bv, gamma):
    offs, taps = _fold_taps(np.asarray(w_ms, np.float32))
    x = np.asarray(x, np.float32)
    bsum = np.asarray(b_ms, np.float32).sum(0)
    gamma_f = float(np.asarray(gamma))
    bv = np.asarray(bv, np.float32)

    w25 = np.empty((128, 25, 2, 2, 128), np.float32)
    for t, off in enumerate(offs):
        # taps[off] is [co, ci]; lhsT wants [ci_lo, t, kt, m, co_lo]
        wt = taps[off].T.reshape(2, 128, 2, 128)   # [kt, ci_lo, m, co_lo]
        w25[:, t] = wt.transpose(1, 0, 2, 3)       # [ci_lo, kt, m, co_lo]
    w25 = (w25 * SW).astype(F8)
    wq8 = (np.asarray(wq, np.float32).T.reshape(2, 128, CQ) * SW) \
        .astype(F8).transpose(1, 0, 2).copy()      # [ci_lo, kt, CQ]
    wk8 = (np.asarray(wk, np.float32).T.reshape(2, 128, CQ) * SW) \
        .astype(F8).transpose(1, 0, 2).copy()
    wv8 = (np.asarray(wv, np.float32).T.reshape(2, 128, 256) * SW) \
        .astype(F8).transpose(1, 0, 2).copy()
    bq_a = np.ascontiguousarray(np.asarray(bq, np.float32).reshape(CQ, 1))
    bk_a = np.ascontiguousarray(np.asarray(bk, np.float32).reshape(CQ, 1))
    bsum_a = np.ascontiguousarray(bsum.reshape(2, 128, 1))

    in_maps = []
    for core in range(NCORES):
        b, g = core // 2, core % 2
        h0 = g * HC
        xp = np.zeros((2, 128, HP, WP), np.float32)
        xs = x[b, :, max(0, h0 - 3):h0 + HC + 3, :]     # rows with halo
        r0 = 3 if h0 == 0 else 0
        xp[:, :, r0:r0 + xs.shape[1], 3:3 + W] = \
            xs.reshape(2, 128, xs.shape[1], W)
        mask01 = np.ones((96, NPOS), np.float32)
        for h in range(HC):
            mask01[h0 + h, np.arange(96) * HC + h] = 0.0
        xresT = (x[b, :, h0:h0 + HC, :] + gamma_f * bv[:, None, None]) \
            .transpose(2, 1, 0).copy()                  # [w, h, c]
        in_maps.append({
            "xpad": xp.transpose(1, 0, 2, 3).astype(F8).copy(),
            "w25": w25, "wq8": wq8, "wk8": wk8, "wv8": wv8,
            "bq": bq_a, "bk": bk_a, "bsum": bsum_a,
            "mask01": mask01.astype(BF16),
            "xresT": np.ascontiguousarray(xresT.astype(np.float32)),
        })
    return in_maps, gamma_f, offs


def run(inputs, trace=False):
    from concourse.bass_utils import run_bass_kernel_spmd
    in_maps, gamma_f, offs = _prepare_inputs(**inputs)
    nc = _build_program(gamma_f, offs)
    res = run_bass_kernel_spmd(nc, in_maps, list(range(NCORES)), trace=trace)
    out = np.empty((B, C, H, W), np.float32)
    for core in range(NCORES):
        b, g = core // 2, core % 2
        r = np.asarray(res.results[core]["out"])        # [w, h, c]
        out[b, :, g * HC:(g + 1) * HC, :] = r.transpose(2, 1, 0)
    return out, res


def kernel(**inputs) -> np.ndarray:
    out, _ = run(inputs, trace=False)
    return out


# revision 15
# speedup vs baseline: 1.6906x; 1.0399x over previous
"""CrissCrossAttention (multi-scale dilated conv + criss-cross axial attention)
Trainium2 Bass/Tile kernel, 8 NeuronCores.

Sharding: 8 cores = 4 batch samples x 2 H-halves.

v3 design:
 - conv as 25 folded taps in fp8e4 with DoubleRow (K=256 per matmul), flat
   416-col rhs runs spanning 4 padded rows (halo cols never read back).
 - all projections (q/k/vT) in fp8 DoubleRow off an fp8 ms tensor.
 - energies computed TRANSPOSED (source index on partitions) so no per-line
   transposes are needed; exp batched 5-10 lines per scalar-engine call.
 - softmax denominators ride along as a ones-column appended to the vT
   operands of the apply matmuls; joint normalization deferred to the tail.
 - pair exchange: vT (fp8) first then k (bf16) via AllGather; a tiny warmup
   collective during conv absorbs the CC-engine startup latency.
 - 16B-aligned strides (272/264) for all hot attention tiles.
 - col->row layout merge via chunked DRAM bounce overlapped with col apply.
 - output kept in [w, h, c] layout on chip; host transposes for free.
"""

import numpy as np
import ml_dtypes

BF16 = ml_dtypes.bfloat16
F8 = ml_dtypes.float8_e4m3

B, C, H, W = 4, 256, 96, 96
CQ = 32
HC = 48              # rows per core
NPOS = HC * W        # 4608 positions per core
HP, WP = 58, 104     # padded slab: 3+48+3 halo rows +4 slack, 3+96+5 cols
NCORES = 8
SW = 64.0            # weight scale for fp8
SV = 32.0            # v scale for fp8
VS = 272             # padded stride of vT tiles (16B aligned, >=257)
AS = 264             # padded stride of acc tiles (bf16 -> 528B, 16B aligned)

NROW = 4             # image rows per conv N-tile
CONV_N = NROW * WP   # 416 flat cols per conv matmul (incl junk)
NT_PROJ = 9
PROJ_N = 512
RG = [[0, 1], [2, 3], [4, 5], [6, 7]]


def _fold_taps(w_ms):
    taps = {}
    for i, d in enumerate((1, 2, 3)):
        for iy in range(3):
            for ix in range(3):
                off = ((iy - 1) * d, (ix - 1) * d)
                if off in taps:
                    taps[off] = taps[off] + w_ms[i][:, :, iy, ix]
                else:
                    taps[off] = w_ms[i][:, :, iy, ix].copy()
    offs = sorted(taps)
    assert len(offs) == 25
    return offs, taps


def _build_program(gamma_f, offs):
    import concourse.mybir as mybir
    import concourse.tile as tile
    from concourse import bacc
    from concourse.masks import make_identity

    dt = mybir.dt
    DR = mybir.MatmulPerfMode.DoubleRow
    nc = bacc.Bacc("TRN2", target_bir_lowering=False, debug=False,
                   num_devices=NCORES)

    xpad_d = nc.dram_tensor("xpad", [128, 2, HP, WP], dt.float8e4, kind="ExternalInput")
    w25_d = nc.dram_tensor("w25", [128, 25, 2, 2, 128], dt.float8e4, kind="ExternalInput")
    wq_d = nc.dram_tensor("wq8", [128, 2, CQ], dt.float8e4, kind="ExternalInput")
    wk_d = nc.dram_tensor("wk8", [128, 2, CQ], dt.float8e4, kind="ExternalInput")
    wv_d = nc.dram_tensor("wv8", [128, 2, 256], dt.float8e4, kind="ExternalInput")
    bq_d = nc.dram_tensor("bq", [CQ, 1], dt.float32, kind="ExternalInput")
    bk_d = nc.dram_tensor("bk", [CQ, 1], dt.float32, kind="ExternalInput")
    bsum_d = nc.dram_tensor("bsum", [2, 128, 1], dt.float32, kind="ExternalInput")
    mask_d = nc.dram_tensor("mask01", [96, NPOS], dt.bfloat16, kind="ExternalInput")
    xres_d = nc.dram_tensor("xresT", [96, HC, 256], dt.float32, kind="ExternalInput")
    out_d = nc.dram_tensor("out", [96, HC, 256], dt.float32, kind="ExternalOutput")

    with tile.TileContext(nc) as tc:
        with (
            tc.tile_pool(name="const", bufs=1) as constp,
            tc.tile_pool(name="dram", bufs=1, space="DRAM") as dramp,
            tc.tile_pool(name="persist", bufs=1) as pp,
        ):
            # ---- constants ----
            id48 = constp.tile([HC, HC], dt.bfloat16, tag="id48", name="id48")
            make_identity(nc, id48)
            bq_sb = constp.tile([CQ, 1], dt.float32, tag="bq", name="bq_sb")
            nc.sync.dma_start(out=bq_sb, in_=bq_d[:])
            bk_sb = constp.tile([CQ, 1], dt.float32, tag="bk", name="bk_sb")
            nc.sync.dma_start(out=bk_sb, in_=bk_d[:])
            bsum_sb = [constp.tile([128, 1], dt.float32, tag=f"bs{m}", name=f"bsum{m}")
                       for m in range(2)]
            for m in range(2):
                nc.sync.dma_start(out=bsum_sb[m], in_=bsum_d[m])

            # ---- persistent tensors ----
            kf = pp.tile([CQ, 2, NPOS], dt.bfloat16, tag="kf", name="kf")
            q_sb = pp.tile([CQ, NPOS], dt.bfloat16, tag="q", name="q_sb")
            vTa = pp.tile([96, HC, VS], dt.float8e4, tag="vTa", name="vTa")
            vTb = pp.tile([96, 96, VS], dt.float8e4, tag="vTb", name="vTb")
            accR = pp.tile([96, HC, AS], dt.bfloat16, tag="accR", name="accR")
            accC = pp.tile([HC, 96, AS], dt.bfloat16, tag="accC", name="accC")
            recipD = pp.tile([96, HC], dt.float32, tag="rD", name="recipD")

            # ones columns for the denominator trick
            nc.vector.memset(vTa[:, :, 256], 1.0)
            nc.vector.memset(vTb[:, :, 256], 1.0)

            # ---- dram bounce buffers ----
            pack_k = dramp.tile([CQ, NPOS], dt.bfloat16, tag="pk", name="pack_k")
            pack_v = dramp.tile([HC, 96, 256], dt.float8e4, tag="pv", name="pack_v")
            gath_k = dramp.tile([2, CQ, NPOS], dt.bfloat16, tag="gk", name="gath_k")
            gath_v = dramp.tile([2, HC, 96, 256], dt.float8e4, tag="gv", name="gath_v")
            bC_d = dramp.tile([HC, 96, 256], dt.bfloat16, tag="bC", name="bC_d")
            warm_i = dramp.tile([1, 48], dt.bfloat16, tag="wi", name="warm_i")
            warm_o = dramp.tile([2, 1, 48], dt.bfloat16, tag="wo", name="warm_o")

            # warmup collective: absorbs the ~11us CC startup latency while
            # the conv runs.
            nc.gpsimd.dma_start(out=warm_i[:], in_=id48[0:1, 0:48])
            nc.gpsimd.collective_compute(
                "AllGather", mybir.AluOpType.bypass, replica_groups=RG,
                ins=[warm_i[:]], outs=[warm_o[:]])

            # ================= Phase 1: conv (25 taps, fp8 DoubleRow) ========
            msp_ctx = tc.tile_pool(name="msp", bufs=1)
            msp = msp_ctx.__enter__()
            ms8 = msp.tile([128, 2, NPOS], dt.float8e4, tag="ms8", name="ms8")
            with (
                tc.tile_pool(name="xw", bufs=1) as xwp,
                tc.tile_pool(name="cvps", bufs=1, space="PSUM") as cvps,
            ):
                w25_sb = xwp.tile([128, 25, 2, 2, 128], dt.float8e4, tag="wt",
                                  name="w25_sb")
                nc.gpsimd.dma_start(out=w25_sb[:, 0:13], in_=w25_d[:, 0:13])
                nc.gpsimd.dma_start(out=w25_sb[:, 13:25], in_=w25_d[:, 13:25])
                xpad_sb = xwp.tile([128, 2, HP, WP], dt.float8e4, tag="xp",
                                   name="xpad_sb")
                nc.sync.dma_start(out=xpad_sb[:, :, 0:26], in_=xpad_d[:, :, 0:26])
                nc.sync.dma_start(out=xpad_sb[:, :, 26:HP], in_=xpad_d[:, :, 26:HP])
                xflat = xpad_sb.rearrange("p k h w -> p k (h w)")

                for g in range(3):      # 3 groups of (2m x 4j) psum tiles
                    P = [[cvps.tile([128, CONV_N], dt.float32, tag=f"cv{m}{j}",
                                    name=f"P{g}{m}{j}", bufs=1)
                          for j in range(4)] for m in range(2)]
                    for t in range(25):
                        dy, dx = offs[t]
                        for m in range(2):
                            lhsT = w25_sb[:, t, :, m, :]
                            for j in range(4):
                                nj = g * 4 + j
                                base = (nj * NROW + 3 + dy) * WP + 3 + dx
                                rhs = xflat[:, :, base:base + CONV_N]
                                nc.tensor.matmul(P[m][j], lhsT, rhs,
                                                 start=(t == 0), stop=(t == 24),
                                                 perf_mode=DR)
                    for m in range(2):
                        for j in range(4):
                            nj = g * 4 + j
                            dst = ms8[:, m, nj * NROW * W:(nj + 1) * NROW * W]
                            nc.vector.tensor_scalar(
                                out=dst.rearrange("p (r c) -> p r c", c=W),
                                in0=P[m][j].rearrange("p (r c) -> p r c", c=WP)[:, :, 0:W],
                                scalar1=1.0 / SW, scalar2=bsum_sb[m],
                                op0=mybir.AluOpType.mult,
                                op1=mybir.AluOpType.add)

            # ================= Phase 2: projections (fp8 DoubleRow) =========
            with (
                tc.tile_pool(name="wproj", bufs=1) as wpp,
                tc.tile_pool(name="pjps", bufs=1, space="PSUM") as pjps,
            ):
                wv_sb = wpp.tile([128, 2, 256], dt.float8e4, tag="wv", name="wv_sb")
                nc.sync.dma_start(out=wv_sb, in_=wv_d[:])
                wq_sb = wpp.tile([128, 2, CQ], dt.float8e4, tag="wq", name="wq_sb")
                nc.sync.dma_start(out=wq_sb, in_=wq_d[:])
                wk_sb = wpp.tile([128, 2, CQ], dt.float8e4, tag="wk", name="wk_sb")
                nc.sync.dma_start(out=wk_sb, in_=wk_d[:])

                # vT first: its exchange is the big one, start it earliest.
                for h in range(HC):
                    pv = pjps.tile([96, 256], dt.float32, tag="pjv",
                                   name=f"pv{h}", bufs=4)
                    nc.tensor.matmul(pv, ms8[:, :, h * W:(h + 1) * W], wv_sb,
                                     start=True, stop=True, perf_mode=DR)
                    nc.vector.tensor_scalar_mul(out=vTa[:, h, 0:256], in0=pv,
                                                scalar1=SV / SW)
                nc.gpsimd.dma_start(
                    out=pack_v[:].rearrange("h w c -> w h c"),
                    in_=vTa[:, :, 0:256])
                nc.gpsimd.collective_compute(
                    "AllGather", mybir.AluOpType.bypass, replica_groups=RG,
                    ins=[pack_v[:]], outs=[gath_v[:]])

                # k projection; own k lands in plane 0 (row attention reads it
                # there before the gather overwrites both planes).
                for n in range(NT_PROJ):
                    sl = slice(n * PROJ_N, (n + 1) * PROJ_N)
                    pk = pjps.tile([CQ, PROJ_N], dt.float32, tag="pjk",
                                   name=f"pk{n}", bufs=2)
                    nc.tensor.matmul(pk, wk_sb, ms8[:, :, sl], start=True,
                                     stop=True, perf_mode=DR)
                    nc.vector.tensor_scalar(out=kf[:, 0, sl], in0=pk,
                                            scalar1=1.0 / SW, scalar2=bk_sb,
                                            op0=mybir.AluOpType.mult,
                                            op1=mybir.AluOpType.add)
                nc.gpsimd.dma_start(out=pack_k[:], in_=kf[:, 0, :])
                nc.gpsimd.collective_compute(
                    "AllGather", mybir.AluOpType.bypass, replica_groups=RG,
                    ins=[pack_k[:]], outs=[gath_k[:]])

                # q projection
                for n in range(NT_PROJ):
                    sl = slice(n * PROJ_N, (n + 1) * PROJ_N)
                    pq = pjps.tile([CQ, PROJ_N], dt.float32, tag="pjq",
                                   name=f"pq{n}", bufs=2)
                    nc.tensor.matmul(pq, wq_sb, ms8[:, :, sl], start=True,
                                     stop=True, perf_mode=DR)
                    nc.vector.tensor_scalar(out=q_sb[:, sl], in0=pq,
                                            scalar1=1.0 / SW, scalar2=bq_sb,
                                            op0=mybir.AluOpType.mult,
                                            op1=mybir.AluOpType.add)

            msp_ctx.__exit__(None, None, None)

            # ============ Phase 3: row attention (fully local) ==============
            HB = [5] * 9 + [3]          # 48 h in blocks of 5 (plus tail 3)
            with (
                tc.tile_pool(name="ph3p", bufs=1) as ph3p,
                tc.tile_pool(name="rps", bufs=1, space="PSUM") as rps,
            ):
                attWT = ph3p.tile([96, NPOS], dt.bfloat16, tag="awt",
                                  name="attWT")
                h = 0
                for nb in HB:
                    pew = rps.tile([96, nb * 96], dt.float32, tag="pew",
                                   name=f"pew{h}", bufs=2)
                    for i in range(nb):
                        sl = slice((h + i) * 96, (h + i + 1) * 96)
                        nc.tensor.matmul(pew[:, i * 96:(i + 1) * 96],
                                         kf[:, 0, sl], q_sb[:, sl],
                                         start=True, stop=True)
                    nc.scalar.activation(
                        out=attWT[:, h * 96:(h + nb) * 96], in_=pew,
                        func=mybir.ActivationFunctionType.Exp)
                    h += nb
                for h in range(HC):
                    po = rps.tile([96, 257], dt.float32, tag="po",
                                  name=f"po{h}", bufs=4)
                    nc.tensor.matmul(po, attWT[:, h * 96:(h + 1) * 96],
                                     vTa[:, h, 0:257], start=True, stop=True)
                    nc.vector.tensor_copy(out=accR[:, h, 0:257], in_=po)

            # ============ Phase 4: column attention (needs exchange) ========
            q3 = q_sb.rearrange("p (h w) -> p h w", w=W)
            kf4 = kf.rearrange("p g (h w) -> p g h w", w=W)
            with (
                tc.tile_pool(name="ph4p", bufs=1) as ph4p,
                tc.tile_pool(name="cps", bufs=1, space="PSUM") as cps,
            ):
                attHT = ph4p.tile([96, NPOS], dt.bfloat16, tag="aht",
                                  name="attHT")
                mask_sb = ph4p.tile([96, NPOS], dt.bfloat16, tag="msk",
                                    name="mask_sb")
                nc.scalar.dma_start(out=mask_sb, in_=mask_d[:])
                # partner k -> both planes of kf (group rank == global half)
                for gi in range(2):
                    nc.scalar.dma_start(out=kf[:, gi, :], in_=gath_k[gi])
                # full-H vT for col apply, partition = global H'; split across
                # queues and w-halves so the pieces run in parallel.
                for gi in range(2):
                    eng = nc.sync if gi == 0 else nc.scalar
                    for wh in range(2):
                        eng.dma_start(
                            out=vTb[gi * HC:(gi + 1) * HC,
                                    wh * HC:(wh + 1) * HC, 0:256],
                            in_=gath_v[gi, :, wh * HC:(wh + 1) * HC, :])

                WB = [10] * 9 + [6]     # 96 w in blocks of 10 (tail 6)
                w = 0
                for nb in WB:
                    peh = cps.tile([96, nb * HC], dt.float32, tag="peh",
                                   name=f"peh{w}", bufs=2)
                    for i in range(nb):
                        nc.tensor.matmul(peh[:, i * HC:(i + 1) * HC],
                                         kf4[:, :, :, w + i], q3[:, :, w + i],
                                         start=True, stop=True)
                    nc.scalar.activation(
                        out=attHT[:, w * HC:(w + nb) * HC], in_=peh,
                        func=mybir.ActivationFunctionType.Exp)
                    # zero the masked diagonal for this block right away
                    nc.vector.tensor_mul(
                        out=attHT[:, w * HC:(w + nb) * HC],
                        in0=attHT[:, w * HC:(w + nb) * HC],
                        in1=mask_sb[:, w * HC:(w + nb) * HC])
                    w += nb
                for wb in range(8):     # blocks of 12 w + chunked bounce-out
                    for i in range(12):
                        w = wb * 12 + i
                        po2 = cps.tile([HC, 257], dt.float32, tag="po2",
                                       name=f"po2{w}", bufs=4)
                        nc.tensor.matmul(po2, attHT[:, w * HC:(w + 1) * HC],
                                         vTb[:, w, 0:257], start=True, stop=True)
                        nc.vector.tensor_copy(out=accC[:, w, 0:257], in_=po2)
                    nc.gpsimd.dma_start(
                        out=bC_d[:, wb * 12:(wb + 1) * 12, :],
                        in_=accC[:, wb * 12:(wb + 1) * 12, 0:256])

            # ============ Phase 5: merge, normalize, residual, out ==========
            with (
                tc.tile_pool(name="fin", bufs=1) as finp,
                tc.tile_pool(name="fps", bufs=1, space="PSUM") as fps,
            ):
                # joint denominator: D = rowD + colD^T
                ptD = fps.tile([96, HC], dt.bfloat16, tag="ptD", name="ptD",
                               bufs=1)
                nc.tensor.transpose(ptD, accC[:, :, 256], id48)
                Dt = finp.tile([96, HC], dt.float32, tag="Dt", name="Dt")
                nc.vector.tensor_add(out=Dt, in0=accR[:, :, 256], in1=ptD)
                nc.vector.tensor_scalar_mul(out=Dt, in0=Dt,
                                            scalar1=SV / gamma_f)
                nc.vector.reciprocal(out=recipD, in_=Dt)

                NH = 12
                for cidx in range(HC // NH):
                    hsl = slice(cidx * NH, (cidx + 1) * NH)
                    xr = finp.tile([96, NH, 256], dt.float32, tag="xr",
                                   name=f"xr{cidx}", bufs=2)
                    nc.sync.dma_start(out=xr, in_=xres_d[:, hsl, :])
                    aCw = finp.tile([96, NH, 256], dt.bfloat16, tag="aCw",
                                    name=f"aCw{cidx}", bufs=2)
                    nc.scalar.dma_start(
                        out=aCw,
                        in_=bC_d[:, :, :].rearrange("h w c -> w h c")[:, hsl, :])
                    nc.vector.tensor_add(out=aCw, in0=aCw,
                                         in1=accR[:, hsl, 0:256])
                    for i in range(NH):
                        h = cidx * NH + i
                        if i % 2 == 0:
                            nc.vector.tensor_scalar_mul(
                                out=aCw[:, i, :], in0=aCw[:, i, :],
                                scalar1=recipD[:, h:h + 1])
                        else:
                            nc.scalar.activation(
                                out=aCw[:, i, :], in_=aCw[:, i, :],
                                func=mybir.ActivationFunctionType.Copy,
                                scale=recipD[:, h:h + 1])
                    fo = finp.tile([96, NH, 256], dt.float32, tag="fo",
                                   name=f"fo{cidx}", bufs=2)
                    nc.vector.tensor_add(out=fo, in0=aCw, in1=xr)
                    nc.sync.dma_start(out=out_d[:, hsl, :], in_=fo)

    nc.compile()
    return nc


def _prepare_inputs(x, w_ms, b_ms, wq, bq, wk, bk, wv, bv, gamma):
    offs, taps = _fold_taps(np.asarray(w_ms, np.float32))
    x = np.asarray(x, np.float32)
    bsum = np.asarray(b_ms, np.float32).sum(0)
    gamma_f = float(np.asarray(gamma))
    bv = np.asarray(bv, np.float32)

    w25 = np.empty((128, 25, 2, 2, 128), np.float32)
    for t, off in enumerate(offs):
        # taps[off] is [co, ci]; lhsT wants [ci_lo, t, kt, m, co_lo]
        wt = taps[off].T.reshape(2, 128, 2, 128)   # [kt, ci_lo, m, co_lo]
        w25[:, t] = wt.transpose(1, 0, 2, 3)       # [ci_lo, kt, m, co_lo]
    w25 = (w25 * SW).astype(F8)
    wq8 = (np.asarray(wq, np.float32).T.reshape(2, 128, CQ) * SW) \
        .astype(F8).transpose(1, 0, 2).copy()      # [ci_lo, kt, CQ]
    wk8 = (np.asarray(wk, np.float32).T.reshape(2, 128, CQ) * SW) \
        .astype(F8).transpose(1, 0, 2).copy()
    wv8 = (np.asarray(wv, np.float32).T.reshape(2, 128, 256) * SW) \
        .astype(F8).transpose(1, 0, 2).copy()
    bq_a = np.ascontiguousarray(np.asarray(bq, np.float32).reshape(CQ, 1))
    bk_a = np.ascontiguousarray(np.asarray(bk, np.float32).reshape(CQ, 1))
    bsum_a = np.ascontiguousarray(bsum.reshape(2, 128, 1))

    in_maps = []
    for core in range(NCORES):
        b, g = core // 2, core % 2
        h0 = g * HC
        xp = np.zeros((2, 128, HP, WP), np.float32)
        xs = x[b, :, max(0, h0 - 3):h0 + HC + 3, :]     # rows with halo
        r0 = 3 if h0 == 0 else 0
        xp[:, :, r0:r0 + xs.shape[1], 3:3 + W] = \
            xs.reshape(2, 128, xs.shape[1], W)
        mask01 = np.ones((96, NPOS), np.float32)
        for h in range(HC):
            mask01[h0 + h, np.arange(96) * HC + h] = 0.0
        xresT = (x[b, :, h0:h0 + HC, :] + gamma_f * bv[:, None, None]) \
            .transpose(2, 1, 0).copy()                  # [w, h, c]
        in_maps.append({
            "xpad": xp.transpose(1, 0, 2, 3).astype(F8).copy(),
            "w25": w25, "wq8": wq8, "wk8": wk8, "wv8": wv8,
            "bq": bq_a, "bk": bk_a, "bsum": bsum_a,
            "mask01": mask01.astype(BF16),
            "xresT": np.ascontiguousarray(xresT.astype(np.float32)),
        })
    return in_maps, gamma_f, offs


def run(inputs, trace=False):
    from concourse.bass_utils import run_bass_kernel_spmd
    in_maps, gamma_f, offs = _prepare_inputs(**inputs)
    nc = _build_program(gamma_f, offs)
    res = run_bass_kernel_spmd(nc, in_maps, list(range(NCORES)), trace=trace)
    out = np.empty((B, C, H, W), np.float32)
    for core in range(NCORES):
        b, g = core // 2, core % 2
        r = np.asarray(res.results[core]["out"])        # [w, h, c]
        out[b, :, g * HC:(g + 1) * HC, :] = r.transpose(2, 1, 0)
    return out, res


def kernel(**inputs) -> np.ndarray:
    out, _ = run(inputs, trace=False)
    return out


# revision 17
# speedup vs baseline: 1.7536x; 1.0373x over previous
"""CrissCrossAttention (multi-scale dilated conv + criss-cross axial attention)
Trainium2 Bass/Tile kernel, 8 NeuronCores.

Sharding: 8 cores = 4 batch samples x 2 H-halves.

v3 design:
 - conv as 25 folded taps in fp8e4 with DoubleRow (K=256 per matmul), flat
   416-col rhs runs spanning 4 padded rows (halo cols never read back).
 - all projections (q/k/vT) in fp8 DoubleRow off an fp8 ms tensor.
 - energies computed TRANSPOSED (source index on partitions) so no per-line
   transposes are needed; exp batched 5-10 lines per scalar-engine call.
 - softmax denominators ride along as a ones-column appended to the vT
   operands of the apply matmuls; joint normalization deferred to the tail.
 - pair exchange: vT (fp8) first then k (bf16) via AllGather; a tiny warmup
   collective during conv absorbs the CC-engine startup latency.
 - 16B-aligned strides (272/264) for all hot attention tiles.
 - col->row layout merge via chunked DRAM bounce overlapped with col apply.
 - output kept in [w, h, c] layout on chip; host transposes for free.
"""

import numpy as np
import ml_dtypes

BF16 = ml_dtypes.bfloat16
F8 = ml_dtypes.float8_e4m3

B, C, H, W = 4, 256, 96, 96
CQ = 32
HC = 48              # rows per core
NPOS = HC * W        # 4608 positions per core
HP, WP = 58, 104     # padded slab: 3+48+3 halo rows +4 slack, 3+96+5 cols
NCORES = 8
SW = 64.0            # weight scale for fp8
SV = 32.0            # v scale for fp8
VS = 272             # padded stride of vT tiles (16B aligned, >=257)
AS = 264             # padded stride of acc tiles (bf16 -> 528B, 16B aligned)

NROW = 4             # image rows per conv N-tile
CONV_N = NROW * WP   # 416 flat cols per conv matmul (incl junk)
NT_PROJ = 9
PROJ_N = 512
RG = [[0, 1], [2, 3], [4, 5], [6, 7]]


def _fold_taps(w_ms):
    taps = {}
    for i, d in enumerate((1, 2, 3)):
        for iy in range(3):
            for ix in range(3):
                off = ((iy - 1) * d, (ix - 1) * d)
                if off in taps:
                    taps[off] = taps[off] + w_ms[i][:, :, iy, ix]
                else:
                    taps[off] = w_ms[i][:, :, iy, ix].copy()
    offs = sorted(taps)
    assert len(offs) == 25
    return offs, taps


def _build_program(gamma_f, offs):
    import concourse.mybir as mybir
    import concourse.tile as tile
    from concourse import bacc
    from concourse.masks import make_identity

    dt = mybir.dt
    DR = mybir.MatmulPerfMode.DoubleRow
    nc = bacc.Bacc("TRN2", target_bir_lowering=False, debug=False,
                   num_devices=NCORES)

    xpad_d = nc.dram_tensor("xpad", [128, 2, HP, WP], dt.float8e4, kind="ExternalInput")
    w25_d = nc.dram_tensor("w25", [128, 25, 2, 2, 128], dt.float8e4, kind="ExternalInput")
    wq_d = nc.dram_tensor("wq8", [128, 2, CQ], dt.float8e4, kind="ExternalInput")
    wk_d = nc.dram_tensor("wk8", [128, 2, CQ], dt.float8e4, kind="ExternalInput")
    wv_d = nc.dram_tensor("wv8", [128, 2, 256], dt.float8e4, kind="ExternalInput")
    bq_d = nc.dram_tensor("bq", [CQ, 1], dt.float32, kind="ExternalInput")
    bk_d = nc.dram_tensor("bk", [CQ, 1], dt.float32, kind="ExternalInput")
    bsum_d = nc.dram_tensor("bsum", [2, 128, 1], dt.float32, kind="ExternalInput")
    mask_d = nc.dram_tensor("mask01", [96, NPOS], dt.bfloat16, kind="ExternalInput")
    xres_d = nc.dram_tensor("xresT", [96, HC, 256], dt.float32, kind="ExternalInput")
    out_d = nc.dram_tensor("out", [96, HC, 256], dt.float32, kind="ExternalOutput")

    with tile.TileContext(nc) as tc:
        with (
            tc.tile_pool(name="const", bufs=1) as constp,
            tc.tile_pool(name="dram", bufs=1, space="DRAM") as dramp,
            tc.tile_pool(name="persist", bufs=1) as pp,
        ):
            # ---- constants ----
            id48 = constp.tile([HC, HC], dt.bfloat16, tag="id48", name="id48")
            make_identity(nc, id48)
            bq_sb = constp.tile([CQ, 1], dt.float32, tag="bq", name="bq_sb")
            nc.scalar.dma_start(out=bq_sb, in_=bq_d[:])
            bk_sb = constp.tile([CQ, 1], dt.float32, tag="bk", name="bk_sb")
            nc.scalar.dma_start(out=bk_sb, in_=bk_d[:])
            bsum_sb = [constp.tile([128, 1], dt.float32, tag=f"bs{m}", name=f"bsum{m}")
                       for m in range(2)]
            for m in range(2):
                nc.scalar.dma_start(out=bsum_sb[m], in_=bsum_d[m])

            # ---- persistent tensors ----
            kf = pp.tile([CQ, 2, NPOS], dt.bfloat16, tag="kf", name="kf")
            k_own = pp.tile([CQ, NPOS], dt.bfloat16, tag="ko", name="k_own")
            q_sb = pp.tile([CQ, NPOS], dt.bfloat16, tag="q", name="q_sb")
            vTa = pp.tile([96, HC, VS], dt.float8e4, tag="vTa", name="vTa")
            vTb = pp.tile([96, 96, VS], dt.float8e4, tag="vTb", name="vTb")
            accR = pp.tile([96, HC, AS], dt.bfloat16, tag="accR", name="accR")
            accC = pp.tile([HC, 96, AS], dt.bfloat16, tag="accC", name="accC")
            recipD = pp.tile([96, HC], dt.float32, tag="rD", name="recipD")

            # ones columns for the denominator trick
            nc.vector.memset(vTa[:, :, 256], 1.0)
            nc.vector.memset(vTb[:, :, 256], 1.0)

            # ---- dram bounce buffers ----
            pack_k = dramp.tile([CQ, NPOS], dt.bfloat16, tag="pk", name="pack_k")
            pack_v = dramp.tile([HC, 96, 256], dt.float8e4, tag="pv", name="pack_v")
            gath_k = dramp.tile([2, CQ, NPOS], dt.bfloat16, tag="gk", name="gath_k")
            gath_v = dramp.tile([2, HC, 96, 256], dt.float8e4, tag="gv", name="gath_v")
            bC_d = dramp.tile([HC, 96, 256], dt.bfloat16, tag="bC", name="bC_d")
            warm_i = dramp.tile([1, 48], dt.bfloat16, tag="wi", name="warm_i")
            warm_o = dramp.tile([2, 1, 48], dt.bfloat16, tag="wo", name="warm_o")

            # warmup collective: absorbs the ~11us CC startup latency while
            # the conv runs.
            nc.gpsimd.dma_start(out=warm_i[:], in_=id48[0:1, 0:48])
            nc.gpsimd.collective_compute(
                "AllGather", mybir.AluOpType.bypass, replica_groups=RG,
                ins=[warm_i[:]], outs=[warm_o[:]])

            # ================= Phase 1: conv (25 taps, fp8 DoubleRow) ========
            msp_ctx = tc.tile_pool(name="msp", bufs=1)
            msp = msp_ctx.__enter__()
            ms8 = msp.tile([128, 2, NPOS], dt.float8e4, tag="ms8", name="ms8")
            with (
                tc.tile_pool(name="xw", bufs=1) as xwp,
                tc.tile_pool(name="cvps", bufs=1, space="PSUM") as cvps,
            ):
                w25_sb = xwp.tile([128, 25, 2, 2, 128], dt.float8e4, tag="wt",
                                  name="w25_sb")
                nc.gpsimd.dma_start(out=w25_sb[:, 0:13], in_=w25_d[:, 0:13])
                nc.gpsimd.dma_start(out=w25_sb[:, 13:25], in_=w25_d[:, 13:25])
                xpad_sb = xwp.tile([128, 2, HP, WP], dt.float8e4, tag="xp",
                                   name="xpad_sb")
                nc.sync.dma_start(out=xpad_sb[:, :, 0:26], in_=xpad_d[:, :, 0:26])
                nc.sync.dma_start(out=xpad_sb[:, :, 26:HP], in_=xpad_d[:, :, 26:HP])
                xflat = xpad_sb.rearrange("p k h w -> p k (h w)")

                for g in range(3):      # 3 groups of (2m x 4j) psum tiles
                    P = [[cvps.tile([128, CONV_N], dt.float32, tag=f"cv{m}{j}",
                                    name=f"P{g}{m}{j}", bufs=1)
                          for j in range(4)] for m in range(2)]
                    for t in range(25):
                        dy, dx = offs[t]
                        for m in range(2):
                            lhsT = w25_sb[:, t, :, m, :]
                            for j in range(4):
                                nj = g * 4 + j
                                base = (nj * NROW + 3 + dy) * WP + 3 + dx
                                rhs = xflat[:, :, base:base + CONV_N]
                                nc.tensor.matmul(P[m][j], lhsT, rhs,
                                                 start=(t == 0), stop=(t == 24),
                                                 perf_mode=DR)
                    for m in range(2):
                        for j in range(4):
                            nj = g * 4 + j
                            dst = ms8[:, m, nj * NROW * W:(nj + 1) * NROW * W]
                            nc.vector.tensor_scalar(
                                out=dst.rearrange("p (r c) -> p r c", c=W),
                                in0=P[m][j].rearrange("p (r c) -> p r c", c=WP)[:, :, 0:W],
                                scalar1=1.0 / SW, scalar2=bsum_sb[m],
                                op0=mybir.AluOpType.mult,
                                op1=mybir.AluOpType.add)

            # ================= Phase 2: projections (fp8 DoubleRow) =========
            with (
                tc.tile_pool(name="wproj", bufs=1) as wpp,
                tc.tile_pool(name="pjps", bufs=1, space="PSUM") as pjps,
            ):
                wv_sb = wpp.tile([128, 2, 256], dt.float8e4, tag="wv", name="wv_sb")
                nc.sync.dma_start(out=wv_sb, in_=wv_d[:])
                wq_sb = wpp.tile([128, 2, CQ], dt.float8e4, tag="wq", name="wq_sb")
                nc.sync.dma_start(out=wq_sb, in_=wq_d[:])
                wk_sb = wpp.tile([128, 2, CQ], dt.float8e4, tag="wk", name="wk_sb")
                nc.sync.dma_start(out=wk_sb, in_=wk_d[:])

                # k projection first: its gather is latency-bound and the
                # column energies need it earliest.
                for n in range(NT_PROJ):
                    sl = slice(n * PROJ_N, (n + 1) * PROJ_N)
                    pk = pjps.tile([CQ, PROJ_N], dt.float32, tag="pjk",
                                   name=f"pk{n}", bufs=2)
                    nc.tensor.matmul(pk, wk_sb, ms8[:, :, sl], start=True,
                                     stop=True, perf_mode=DR)
                    nc.vector.tensor_scalar(out=k_own[:, sl], in0=pk,
                                            scalar1=1.0 / SW, scalar2=bk_sb,
                                            op0=mybir.AluOpType.mult,
                                            op1=mybir.AluOpType.add)
                nc.gpsimd.dma_start(out=pack_k[:], in_=k_own[:])
                nc.gpsimd.collective_compute(
                    "AllGather", mybir.AluOpType.bypass, replica_groups=RG,
                    ins=[pack_k[:]], outs=[gath_k[:]])

                # vT projection + the big (bandwidth-bound) exchange.
                for h in range(HC):
                    pv = pjps.tile([96, 256], dt.float32, tag="pjv",
                                   name=f"pv{h}", bufs=4)
                    nc.tensor.matmul(pv, ms8[:, :, h * W:(h + 1) * W], wv_sb,
                                     start=True, stop=True, perf_mode=DR)
                    nc.vector.tensor_scalar_mul(out=vTa[:, h, 0:256], in0=pv,
                                                scalar1=SV / SW)
                nc.gpsimd.dma_start(
                    out=pack_v[:].rearrange("h w c -> w h c"),
                    in_=vTa[:, :, 0:256])
                nc.gpsimd.collective_compute(
                    "AllGather", mybir.AluOpType.bypass, replica_groups=RG,
                    ins=[pack_v[:]], outs=[gath_v[:]])

                # q projection
                for n in range(NT_PROJ):
                    sl = slice(n * PROJ_N, (n + 1) * PROJ_N)
                    pq = pjps.tile([CQ, PROJ_N], dt.float32, tag="pjq",
                                   name=f"pq{n}", bufs=2)
                    nc.tensor.matmul(pq, wq_sb, ms8[:, :, sl], start=True,
                                     stop=True, perf_mode=DR)
                    nc.vector.tensor_scalar(out=q_sb[:, sl], in0=pq,
                                            scalar1=1.0 / SW, scalar2=bq_sb,
                                            op0=mybir.AluOpType.mult,
                                            op1=mybir.AluOpType.add)

            msp_ctx.__exit__(None, None, None)

            # ============ Phase 3: row attention (fully local) ==============
            HB = [5] * 9 + [3]          # 48 h in blocks of 5 (plus tail 3)
            with (
                tc.tile_pool(name="ph3p", bufs=1) as ph3p,
                tc.tile_pool(name="rps", bufs=1, space="PSUM") as rps,
            ):
                attWT = ph3p.tile([96, NPOS], dt.bfloat16, tag="awt",
                                  name="attWT")
                h = 0
                for nb in HB:
                    pew = rps.tile([96, nb * 96], dt.float32, tag="pew",
                                   name=f"pew{h}", bufs=2)
                    for i in range(nb):
                        sl = slice((h + i) * 96, (h + i + 1) * 96)
                        nc.tensor.matmul(pew[:, i * 96:(i + 1) * 96],
                                         k_own[:, sl], q_sb[:, sl],
                                         start=True, stop=True)
                    nc.scalar.activation(
                        out=attWT[:, h * 96:(h + nb) * 96], in_=pew,
                        func=mybir.ActivationFunctionType.Exp)
                    h += nb
                for h in range(HC):
                    po = rps.tile([96, 257], dt.float32, tag="po",
                                  name=f"po{h}", bufs=6)
                    nc.tensor.matmul(po, attWT[:, h * 96:(h + 1) * 96],
                                     vTa[:, h, 0:257], start=True, stop=True)
                    if h % 2 == 0:
                        nc.vector.tensor_copy(out=accR[:, h, 0:257], in_=po)
                    else:
                        nc.scalar.activation(
                            out=accR[:, h, 0:257], in_=po,
                            func=mybir.ActivationFunctionType.Copy)

            # ============ Phase 4: column attention (needs exchange) ========
            q3 = q_sb.rearrange("p (h w) -> p h w", w=W)
            kf4 = kf.rearrange("p g (h w) -> p g h w", w=W)
            with (
                tc.tile_pool(name="ph4p", bufs=1) as ph4p,
                tc.tile_pool(name="cps", bufs=1, space="PSUM") as cps,
            ):
                attHT = ph4p.tile([96, NPOS], dt.bfloat16, tag="aht",
                                  name="attHT")
                mask_sb = ph4p.tile([96, NPOS], dt.bfloat16, tag="msk",
                                    name="mask_sb")
                nc.scalar.dma_start(out=mask_sb, in_=mask_d[:])
                # partner k -> both planes of kf (group rank == global half)
                for gi in range(2):
                    nc.scalar.dma_start(out=kf[:, gi, :], in_=gath_k[gi])
                # full-H vT for col apply, partition = global H'; split across
                # queues and w-halves so the pieces run in parallel.
                for gi in range(2):
                    eng = nc.sync if gi == 0 else nc.scalar
                    for wh in range(2):
                        eng.dma_start(
                            out=vTb[gi * HC:(gi + 1) * HC,
                                    wh * HC:(wh + 1) * HC, 0:256],
                            in_=gath_v[gi, :, wh * HC:(wh + 1) * HC, :])

                WB = [10] * 9 + [6]     # 96 w in blocks of 10 (tail 6)
                w = 0
                for nb in WB:
                    peh = cps.tile([96, nb * HC], dt.float32, tag="peh",
                                   name=f"peh{w}", bufs=2)
                    for i in range(nb):
                        nc.tensor.matmul(peh[:, i * HC:(i + 1) * HC],
                                         kf4[:, :, :, w + i], q3[:, :, w + i],
                                         start=True, stop=True)
                    nc.scalar.activation(
                        out=attHT[:, w * HC:(w + nb) * HC], in_=peh,
                        func=mybir.ActivationFunctionType.Exp)
                    # zero the masked diagonal for this block right away
                    nc.vector.tensor_mul(
                        out=attHT[:, w * HC:(w + nb) * HC],
                        in0=attHT[:, w * HC:(w + nb) * HC],
                        in1=mask_sb[:, w * HC:(w + nb) * HC])
                    w += nb
                for wb in range(8):     # blocks of 12 w + chunked bounce-out
                    for i in range(12):
                        w = wb * 12 + i
                        po2 = cps.tile([HC, 257], dt.float32, tag="po2",
                                       name=f"po2{w}", bufs=6)
                        nc.tensor.matmul(po2, attHT[:, w * HC:(w + 1) * HC],
                                         vTb[:, w, 0:257], start=True, stop=True)
                        if w % 2 == 0:
                            nc.vector.tensor_copy(out=accC[:, w, 0:257], in_=po2)
                        else:
                            nc.scalar.activation(
                                out=accC[:, w, 0:257], in_=po2,
                                func=mybir.ActivationFunctionType.Copy)
                    nc.gpsimd.dma_start(
                        out=bC_d[:, wb * 12:(wb + 1) * 12, :],
                        in_=accC[:, wb * 12:(wb + 1) * 12, 0:256])

            # ============ Phase 5: merge, normalize, residual, out ==========
            with (
                tc.tile_pool(name="fin", bufs=1) as finp,
                tc.tile_pool(name="fps", bufs=1, space="PSUM") as fps,
            ):
                # joint denominator: D = rowD + colD^T
                ptD = fps.tile([96, HC], dt.bfloat16, tag="ptD", name="ptD",
                               bufs=1)
                nc.tensor.transpose(ptD, accC[:, :, 256], id48)
                Dt = finp.tile([96, HC], dt.float32, tag="Dt", name="Dt")
                nc.vector.tensor_add(out=Dt, in0=accR[:, :, 256], in1=ptD)
                nc.vector.tensor_scalar_mul(out=Dt, in0=Dt,
                                            scalar1=SV / gamma_f)
                nc.vector.reciprocal(out=recipD, in_=Dt)

                NH = 8
                for cidx in range(HC // NH):
                    hsl = slice(cidx * NH, (cidx + 1) * NH)
                    xr = finp.tile([96, NH, 256], dt.float32, tag="xr",
                                   name=f"xr{cidx}", bufs=2)
                    nc.sync.dma_start(out=xr, in_=xres_d[:, hsl, :])
                    aCw = finp.tile([96, NH, 256], dt.bfloat16, tag="aCw",
                                    name=f"aCw{cidx}", bufs=2)
                    nc.scalar.dma_start(
                        out=aCw,
                        in_=bC_d[:, :, :].rearrange("h w c -> w h c")[:, hsl, :])
                    if cidx % 2 == 0:
                        nc.gpsimd.tensor_add(out=aCw, in0=aCw,
                                             in1=accR[:, hsl, 0:256])
                    else:
                        nc.vector.tensor_add(out=aCw, in0=aCw,
                                             in1=accR[:, hsl, 0:256])
                    for i in range(NH):
                        h = cidx * NH + i
                        if i % 2 == 0:
                            nc.vector.tensor_scalar_mul(
                                out=aCw[:, i, :], in0=aCw[:, i, :],
                                scalar1=recipD[:, h:h + 1])
                        else:
                            nc.scalar.activation(
                                out=aCw[:, i, :], in_=aCw[:, i, :],
                                func=mybir.ActivationFunctionType.Copy,
                                scale=recipD[:, h:h + 1])
                    fo = finp.tile([96, NH, 256], dt.float32, tag="fo",
                                   name=f"fo{cidx}", bufs=2)
                    if cidx % 2 == 0:
                        nc.vector.tensor_add(out=fo, in0=aCw, in1=xr)
                    else:
                        nc.gpsimd.tensor_add(out=fo, in0=aCw, in1=xr)
                    nc.sync.dma_start(out=out_d[:, hsl, :], in_=fo)

    nc.compile()
    return nc


def _prepare_inputs(x, w_ms, b_ms, wq, bq, wk, bk, wv, bv, gamma):
    offs, taps = _fold_taps(np.asarray(w_ms, np.float32))
    x = np.asarray(x, np.float32)
    bsum = np.asarray(b_ms, np.float32).sum(0)
    gamma_f = float(np.asarray(gamma))
    bv = np.asarray(bv, np.float32)

    w25 = np.empty((128, 25, 2, 2, 128), np.float32)
    for t, off in enumerate(offs):
        # taps[off] is [co, ci]; lhsT wants [ci_lo, t, kt, m, co_lo]
        wt = taps[off].T.reshape(2, 128, 2, 128)   # [kt, ci_lo, m, co_lo]
        w25[:, t] = wt.transpose(1, 0, 2, 3)       # [ci_lo, kt, m, co_lo]
    w25 = (w25 * SW).astype(F8)
    wq8 = (np.asarray(wq, np.float32).T.reshape(2, 128, CQ) * SW) \
        .astype(F8).transpose(1, 0, 2).copy()      # [ci_lo, kt, CQ]
    wk8 = (np.asarray(wk, np.float32).T.reshape(2, 128, CQ) * SW) \
        .astype(F8).transpose(1, 0, 2).copy()
    wv8 = (np.asarray(wv, np.float32).T.reshape(2, 128, 256) * SW) \
        .astype(F8).transpose(1, 0, 2).copy()
    bq_a = np.ascontiguousarray(np.asarray(bq, np.float32).reshape(CQ, 1))
    bk_a = np.ascontiguousarray(np.asarray(bk, np.float32).reshape(CQ, 1))
    bsum_a = np.ascontiguousarray(bsum.reshape(2, 128, 1))

    in_maps = []
    for core in range(NCORES):
        b, g = core // 2, core % 2
        h0 = g * HC
        xp = np.zeros((2, 128, HP, WP), np.float32)
        xs = x[b, :, max(0, h0 - 3):h0 + HC + 3, :]     # rows with halo
        r0 = 3 if h0 == 0 else 0
        xp[:, :, r0:r0 + xs.shape[1], 3:3 + W] = \
            xs.reshape(2, 128, xs.shape[1], W)
        mask01 = np.ones((96, NPOS), np.float32)
        for h in range(HC):
            mask01[h0 + h, np.arange(96) * HC + h] = 0.0
        xresT = (x[b, :, h0:h0 + HC, :] + gamma_f * bv[:, None, None]) \
            .transpose(2, 1, 0).copy()                  # [w, h, c]
        in_maps.append({
            "xpad": xp.transpose(1, 0, 2, 3).astype(F8).copy(),
            "w25": w25, "wq8": wq8, "wk8": wk8, "wv8": wv8,
            "bq": bq_a, "bk": bk_a, "bsum": bsum_a,
            "mask01": mask01.astype(BF16),
            "xresT": np.ascontiguousarray(xresT.astype(np.float32)),
        })
    return in_maps, gamma_f, offs


def run(inputs, trace=False):
    from concourse.bass_utils import run_bass_kernel_spmd
    in_maps, gamma_f, offs = _prepare_inputs(**inputs)
    nc = _build_program(gamma_f, offs)
    res = run_bass_kernel_spmd(nc, in_maps, list(range(NCORES)), trace=trace)
    out = np.empty((B, C, H, W), np.float32)
    for core in range(NCORES):
        b, g = core // 2, core % 2
        r = np.asarray(res.results[core]["out"])        # [w, h, c]
        out[b, :, g * HC:(g + 1) * HC, :] = r.transpose(2, 1, 0)
    return out, res


def kernel(**inputs) -> np.ndarray:
    out, _ = run(inputs, trace=False)
    return out
